# revision 46
# baseline (speedup 1.0000x reference)
"""CosAttention TRN2 kernel: qkv projection + cosine-sim attention.

Sharding: data-parallel over batch (B=8 -> one batch element per core).

v2 design (fused single-phase pipeline):
  - host passes xT/WT in bf16 (halves DMA, kills f32r staging copies)
  - per head-pair j: projection+normalization prep for pair j+1 overlaps
    scores+exp of pair j and PV of pair j-1, so the ACT exp stream
    (~29.5M exps/core, the hard floor) paces the kernel
  - k-side L2 norm folded into exp via per-partition scale AP:
    P = exp(s_raw * rinv_k) -- no k normalize multiply at all
  - q-side norm: gmat-reduce -> ACT sqrt -> reciprocal -> partition-
    broadcast DMA -> one in-place multiply
  - score matmuls for the head pair sit on disjoint 64-partition row
    groups -> concurrent PE execution
  - PV uses vaug (v + ones column) so the softmax denominator falls out
    of the PV matmul; eviction divide runs on the idle GPSIMD engine
    (normalize_recip) instead of DVE reciprocal+mult
"""

import os as _os
import sys

sys.path.insert(0, "/opt/trn_rl_repo")

from contextlib import ExitStack

import numpy as np
import ml_dtypes

import concourse.bass as bass
import concourse.tile as tile
from concourse import bacc, mybir
from concourse.bass_utils import run_bass_kernel_spmd

B, N, C = 8, 1568, 768
H, D = 12, 64
NC3 = 3 * C  # 2304
QK = 2 * C  # 1536
F32 = mybir.dt.float32
BF16 = mybir.dt.bfloat16
F8E3 = mybir.dt.float8e3

NT = 13  # token tiles of 128 (last is 32 wide)
KC = 6  # contraction chunks of 128 over C
QBLKS = [(0, 512), (512, 512), (1024, 512), (1536, 32)]
NHALF = 784

ADD = mybir.AluOpType.add
MULT = mybir.AluOpType.mult
DIV = mybir.AluOpType.divide
AF = mybir.ActivationFunctionType


def _knob(name, dflt):
    return _os.environ.get(name, dflt)


PT_DT = {"bf16": BF16, "f8e3": F8E3}[_knob("PT_DT", "f8e3")]
PVEVICT = _knob("PVEVICT", "gps")  # gps | dve
NORM = _knob("NORM", "lnexp")  # lnexp | newton


def _tw(t):
    return 32 if t == NT - 1 else 128


def _bcast_rows(src_row_ap, nrows, drop_first=True):
    """AP reading one partition row repeated nrows times (partition stride 0)."""
    ap = list(src_row_ap.ap)
    if drop_first:
        ap = ap[1:]
    return bass.AP(
        tensor=src_row_ap.tensor,
        offset=src_row_ap.offset,
        ap=[[0, nrows]] + ap,
    )


def _pin_act_table(arch):
    """Force Exp+Ln to resolve to the one table set containing both, so the
    kernel needs a single ACT_TABLE_LOAD instead of thrashing between the
    exp-only and ln-only sets (~1.5us per reload, mid-exp-stream)."""
    import concourse.hw_specs as hw_specs

    tabs = hw_specs.get_activation_tables(arch)  # cached by reference
    combined = "natural_log_exp_and_others"
    if combined not in tabs:
        return
    for name, fns in tabs.items():
        if name != combined:
            fns.discard(AF.Exp)
            fns.discard(AF.Ln)


def _build():
    nc = bacc.Bacc()
    _pin_act_table(nc.m.arch)
    xT = nc.dram_tensor("xT", [C, N], BF16, kind="ExternalInput")
    WT = nc.dram_tensor("WT", [C, NC3], BF16, kind="ExternalInput")
    bqkv = nc.dram_tensor("bqkv", [NC3], F32, kind="ExternalInput")
    out = nc.dram_tensor("out", [N, C], F32, kind="ExternalOutput")
    # per-query 1/|q| bounced through DRAM to broadcast across partitions
    scr = nc.dram_tensor("scr", [6, 2, N], F32, kind="Internal")

    with ExitStack() as ctx:
        tc = ctx.enter_context(tile.TileContext(nc))
        persist = ctx.enter_context(tc.tile_pool(name="persist", bufs=1))

        xb = persist.tile([128, KC, N], BF16)
        vaug = persist.tile([128, NT, H, D + 1], BF16)
        gmat = persist.tile([128, 2], BF16)  # per-head-half sumsq reducer
        bqk_t = persist.tile([128, 12], F32)  # q/k bias, per m-tile column
        bv_b = persist.tile([128, C], F32)  # v bias broadcast to all partitions

        for c in range(KC):
            nc.sync.dma_start(out=xb[:, c, :], in_=xT[c * 128 : (c + 1) * 128, :])
        nc.sync.dma_start(out=bqk_t, in_=bqkv[0:QK].rearrange("(t p) -> p t", p=128))
        bv_src = bqkv[QK:NC3]
        nc.sync.dma_start(out=bv_b, in_=_bcast_rows(bv_src, 128, drop_first=False))
        nc.vector.memset(gmat, 0.0)
        nc.vector.memset(gmat[0:64, 0:1], 1.0)
        nc.vector.memset(gmat[64:128, 1:2], 1.0)
        # ones in every head's extra column; v eviction overwrites the rest
        nc.vector.memset(vaug, 1.0)

        # ---------------- fused projection + attention ----------------
        with (
            tc.tile_pool(name="wv", bufs=1) as wvp,
            tc.tile_pool(name="wq", bufs=2) as wqp,
            tc.tile_pool(name="qk", bufs=2) as qkp,
            tc.tile_pool(name="sq", bufs=2) as sqp,
            tc.tile_pool(name="bc", bufs=1) as bcp,
            tc.tile_pool(name="rk", bufs=4) as rkp,
            tc.tile_pool(name="kss", bufs=2) as kssp,
            tc.tile_pool(name="pt", bufs=4 if PT_DT == F8E3 else 3) as ptp,
            tc.tile_pool(name="ot", bufs=2) as otp,
            tc.tile_pool(name="povs", bufs=3) as povsp,
            tc.tile_pool(name="sc", bufs=3, space="PSUM") as scp,
            tc.tile_pool(name="po", bufs=1, space="PSUM") as pop,
            tc.tile_pool(name="pj", bufs=1, space="PSUM") as pjp,
        ):
            wv = wvp.tile([128, KC, C], BF16)
            for c in range(KC):
                nc.sync.dma_start(
                    out=wv[:, c, :], in_=WT[c * 128 : (c + 1) * 128, QK:NC3]
                )

            def emit_vproj(t):
                w = _tw(t)
                ps = scp.tile([128, NHALF], F32, tag="sc")
                for c in range(KC):
                    for (b0, bw) in ((0, 512), (512, 256)):
                        nc.tensor.matmul(
                            ps[0:w, b0 : b0 + bw],
                            xb[:, c, t * 128 : t * 128 + w],
                            wv[:, c, b0 : b0 + bw],
                            start=(c == 0),
                            stop=(c == KC - 1),
                        )
                nc.vector.tensor_add(
                    vaug[0:w, t, :, 0:D],
                    ps[0:w, 0:C].rearrange("p (h d) -> p h d", d=D),
                    bv_b[0:w, :].rearrange("p (h d) -> p h d", d=D),
                )
            state = {}

            def prep_steps(j):
                """Projection + normalization for head pair j; list of thunks."""
                st = {}
                steps = []

                def load_wq(m, key):
                    def f():
                        wq = wqp.tile([128, KC, 128], BF16, tag="wq", name=f"wq{m}")
                        nc.sync.dma_start(
                            out=wq,
                            in_=WT[:, m * 128 : (m + 1) * 128].rearrange(
                                "(c p) n -> p c n", p=128
                            ),
                        )
                        st[key] = wq

                    return f

                def proj_chunk(m, key, dstkey, q0, qw, blk, c):
                    # one matmul per step: prep trickles through the PE queue
                    # without ever blocking the score-psum refills ACT needs
                    def f():
                        if blk == 0 and c == 0:
                            st[dstkey] = qkp.tile(
                                [128, N], BF16, tag=dstkey[:2], name=f"{dstkey}{j}"
                            )
                        if c == 0:
                            st[dstkey + "ps"] = pjp.tile([128, 512], F32, tag="pj", name=f"{dstkey}ps{blk}")
                        nc.tensor.matmul(
                            st[dstkey + "ps"][:, 0:qw],
                            st[key][:, c, :],
                            xb[:, c, q0 : q0 + qw],
                            start=(c == 0),
                            stop=(c == KC - 1),
                        )

                    return f

                def proj_evict(m, dstkey, q0, qw):
                    def f():
                        # bias-add while evicting to bf16
                        nc.vector.tensor_scalar(
                            out=st[dstkey][:, q0 : q0 + qw],
                            in0=st[dstkey + "ps"][:, 0:qw],
                            scalar1=bqk_t[:, m : m + 1],
                            scalar2=None,
                            op0=ADD,
                        )

                    return f

                def proj_blk_steps(m, key, dstkey):
                    out = []
                    for blk, (q0, qw) in enumerate(QBLKS):
                        for c in range(KC):
                            out.append(proj_chunk(m, key, dstkey, q0, qw, blk, c))
                        out.append(proj_evict(m, dstkey, q0, qw))
                    return out

                def reduce_rinv(sqtile, dst):
                    # per-token per-head sumsq -> rinv = exp(-0.5*ln(ssq));
                    # Log+Exp share one ACT table set, so no table thrash
                    ps = pjp.tile([128, 512], F32, tag="pj")
                    for t in range(NT):
                        w = _tw(t)
                        nc.tensor.matmul(
                            ps[0:w, 2 * t : 2 * t + 2],
                            sqtile[:, t * 128 : t * 128 + w],
                            gmat,
                            start=True,
                            stop=True,
                        )
                    lg = kssp.tile([128, NT * 2], F32, tag="kss")
                    nc.scalar.activation(out=lg, in_=ps[:, 0 : 2 * NT], func=AF.Ln)
                    nc.scalar.activation(out=dst, in_=lg, func=AF.Exp, scale=-0.5)

                # ---- q side: m = j ----
                steps.append(load_wq(j, "wq_q"))
                steps.extend(proj_blk_steps(j, "wq_q", "qp"))

                def q_reduce_blk(q0, qw, blk):
                    # q-side rinv in [head-row, query] layout: ln on the sumsq
                    # psum chunk, one exp(-0.5*.) over the row at the end
                    def f():
                        if blk == 0:
                            st["sqq"] = sqp.tile(
                                [128, N], BF16, tag="sq", name=f"sqq{j}"
                            )
                            st["lgq"] = bcp.tile(
                                [2, N], F32, tag="lgq", name=f"lgq{j}"
                            )
                        nc.vector.tensor_mul(
                            st["sqq"][:, q0 : q0 + qw],
                            st["qp"][:, q0 : q0 + qw],
                            st["qp"][:, q0 : q0 + qw],
                        )
                        ps = pjp.tile([128, 512], F32, tag="pj")
                        nc.tensor.matmul(
                            ps[0:2, 0:qw],
                            gmat,
                            st["sqq"][:, q0 : q0 + qw],
                            start=True,
                            stop=True,
                        )
                        nc.scalar.activation(
                            out=st["lgq"][:, q0 : q0 + qw], in_=ps[0:2, 0:qw], func=AF.Ln
                        )
                        if blk == len(QBLKS) - 1:
                            rqT = bcp.tile([2, N], F32, tag="rqT", name=f"rqT{j}")
                            nc.scalar.activation(
                                out=rqT, in_=st["lgq"], func=AF.Exp, scale=-0.5
                            )
                            nc.sync.dma_start(out=scr[j], in_=rqT)

                    return f

                for blk, (q0, qw) in enumerate(QBLKS):
                    steps.append(q_reduce_blk(q0, qw, blk))

                def q_bcast():
                    bc = bcp.tile([128, N], F32, tag="bc", name=f"bc{j}")
                    for hb in range(2):
                        nc.sync.dma_start(
                            out=bc[64 * hb : 64 * hb + 64, :],
                            in_=_bcast_rows(scr[j, hb : hb + 1, :], 64),
                        )
                    st["bc"] = bc

                steps.append(q_bcast)

                def q_mul():
                    # in-place: qp becomes the normalized q
                    nc.vector.tensor_mul(st["qp"], st["qp"], st["bc"])

                steps.append(q_mul)

                # ---- k side: m = j + 6 ----
                steps.append(load_wq(j + 6, "wq_k"))
                steps.extend(proj_blk_steps(j + 6, "wq_k", "kp"))

                def k_sq_chunk(q0, qw, blk):
                    def f():
                        if blk == 0:
                            st["sqk"] = sqp.tile(
                                [128, N], BF16, tag="sq", name=f"sqk{j}"
                            )
                        nc.vector.tensor_mul(
                            st["sqk"][:, q0 : q0 + qw],
                            st["kp"][:, q0 : q0 + qw],
                            st["kp"][:, q0 : q0 + qw],
                        )

                    return f

                for blk, (q0, qw) in enumerate(QBLKS):
                    steps.append(k_sq_chunk(q0, qw, blk))

                def k_red_mm(t):
                    def f():
                        if t == 0:
                            st["kps"] = pjp.tile([128, 512], F32, tag="pj", name=f"kps{j}")
                        w = _tw(t)
                        nc.tensor.matmul(
                            st["kps"][0:w, 2 * t : 2 * t + 2],
                            st["sqk"][:, t * 128 : t * 128 + w],
                            gmat,
                            start=True,
                            stop=True,
                        )

                    return f

                for t in range(NT):
                    steps.append(k_red_mm(t))

                def k_rinv():
                    lg = kssp.tile([128, NT * 2], F32, tag="kss")
                    nc.scalar.activation(
                        out=lg, in_=st["kps"][:, 0 : 2 * NT], func=AF.Ln
                    )
                    rk = rkp.tile([128, NT, 2], F32, tag="rk", name=f"rk{j}")
                    nc.scalar.activation(
                        out=rk[:, :, :], in_=lg, func=AF.Exp, scale=-0.5
                    )
                    st["rk"] = rk

                steps.append(k_rinv)

                def finish():
                    state[j] = (st["qp"], st["kp"], st["rk"])

                steps.append(finish)
                return steps

            pts = {}

            def emit_scores(j, hb, t, half):
                # paired emission: the A-head (rows 0-63) and B-head (rows
                # 64-127) chunk matmuls are adjacent in the PE queue and run
                # concurrently on disjoint row groups
                qp, kp, rk = state[j][:3]
                w = _tw(t)
                base = half * NHALF
                scs = []
                for b in range(2):
                    sct = scp.tile(
                        [128, NHALF], F32, tag="sc", name=f"sc{j}_{t}_{half}_{b}"
                    )
                    scs.append(sct)
                for (b0, bw) in ((0, 512), (512, NHALF - 512)):
                    for b in range(2):
                        hp = 64 * b
                        nc.tensor.matmul(
                            scs[b][0:w, b0 : b0 + bw],
                            kp[hp : hp + 64, t * 128 : t * 128 + w],
                            qp[hp : hp + 64, base + b0 : base + b0 + bw],
                            start=True,
                            stop=True,
                        )
                # exp(s * rinv_k): cosine range needs no max subtraction
                for b in range(2):
                    nc.scalar.activation(
                        out=pts[2 * j + b][0:w, t, base : base + NHALF],
                        in_=scs[b][0:w, :],
                        func=AF.Exp,
                        scale=rk[0:w, t, b : b + 1],
                    )

            def emit_pv(j, hb, qt):
                qp, kp, rk = state[j][:3]
                q0 = qt * 128
                qw = _tw(qt)
                po = pop.tile([128, D + 1], F32, tag="po")
                for t in range(NT):
                    w = _tw(t)
                    nc.tensor.matmul(
                        po[0:qw, :],
                        pts[2 * j + hb][0:w, t, q0 : q0 + qw],
                        vaug[0:w, t, 2 * j + hb, :],
                        start=(t == 0),
                        stop=(t == NT - 1),
                    )
                if hb == 0 and qt == 0:
                    st_ot = otp.tile([128, NT, 2, D], F32, tag="ot", name=f"ot{j}")
                    state[j] = (qp, kp, rk, st_ot)
                ot = state[j][3]
                pv = povsp.tile([128, D + 1], F32, tag="povs")
                nc.vector.tensor_copy(pv[0:qw, :], po[0:qw, :])
                if PVEVICT == "gps":
                    nc.gpsimd.normalize_recip(
                        ot[0:qw, qt, hb, :],
                        pv[0:qw, 0:D],
                        pv[0:qw, D : D + 1],
                    )
                else:
                    linv = povsp.tile([128, 1], F32, tag="linv")
                    nc.vector.reciprocal(out=linv[0:qw, :], in_=pv[0:qw, D : D + 1])
                    nc.vector.tensor_scalar(
                        out=ot[0:qw, qt, hb, :],
                        in0=pv[0:qw, 0:D],
                        scalar1=linv[0:qw, :],
                        scalar2=None,
                        op0=MULT,
                    )
                if hb == 1:
                    nc.sync.dma_start(
                        out=out[q0 : q0 + qw, 2 * j * D : (2 * j + 2) * D],
                        in_=ot[0:qw, qt, :, :],
                    )

            # ---- schedule ----
            # v-projection interleaves with prep(0): prep's critical path is
            # DMA/latency-bound, so the v matmuls fill otherwise-idle PE time
            steps0 = prep_steps(0)
            vp = 0
            for i, s in enumerate(steps0):
                s()
                if vp < NT and i % 6 == 5:
                    emit_vproj(vp)
                    vp += 1
            while vp < NT:
                emit_vproj(vp)
                vp += 1
            for j in range(7):
                steps = prep_steps(j + 1) if j + 1 <= 5 else []
                per_t = (len(steps) + NT - 1) // NT if steps else 0
                si = 0
                if j < 6:
                    pts[2 * j] = ptp.tile([128, NT, N], PT_DT, tag="pt", name=f"pt{2*j}")
                    pts[2 * j + 1] = ptp.tile(
                        [128, NT, N], PT_DT, tag="pt", name=f"pt{2*j+1}"
                    )
                for t in range(NT):
                    # fine-grained interleave: PE revisits score matmuls every
                    # ~1us so the 2-buffer score psum pool keeps ACT streaming
                    # both score halves first: ACT gets a 4-exp run-ahead
                    # buffer that covers the PV/prep interlude on the PE FIFO
                    if j < 6:
                        emit_scores(j, 0, t, 0)
                        emit_scores(j, 0, t, 1)
                    if j > 0:
                        emit_pv(j - 1, 0, t)
                        emit_pv(j - 1, 1, t)
                    for _ in range(per_t):
                        if si < len(steps):
                            steps[si]()
                            si += 1
                while si < len(steps):
                    steps[si]()
                    si += 1
                if j > 0:
                    del pts[2 * (j - 1)]
                    del pts[2 * (j - 1) + 1]
                    del state[j - 1]

    nc.compile()
    return nc


_PROGRAM = None


def _get_program():
    global _PROGRAM
    if _PROGRAM is None:
        _PROGRAM = _build()
    return _PROGRAM


_LAST_RESULTS = None


def kernel(x, W_qkv, b_qkv):
    global _LAST_RESULTS
    nc = _get_program()
    xT = np.ascontiguousarray(
        np.transpose(np.asarray(x, np.float32), (0, 2, 1))
    ).astype(ml_dtypes.bfloat16)
    WTh = np.ascontiguousarray(np.asarray(W_qkv, np.float32).T).astype(
        ml_dtypes.bfloat16
    )
    bh = np.ascontiguousarray(np.asarray(b_qkv, np.float32))
    in_maps = [{"xT": xT[b], "WT": WTh, "bqkv": bh} for b in range(B)]
    res = run_bass_kernel_spmd(nc, in_maps, core_ids=list(range(B)))
    _LAST_RESULTS = res
    o = np.stack([np.asarray(res.results[b]["out"]) for b in range(B)], axis=0)
    return np.ascontiguousarray(o.astype(np.float32))


if __name__ == "__main__":
    _build()
    print("build OK")


# revision 48
# speedup vs baseline: 1.0467x; 1.0467x over previous
"""CosAttention TRN2 kernel: qkv projection + cosine-sim attention.

Sharding: data-parallel over batch (B=8 -> one batch element per core).

v2 design (fused single-phase pipeline):
  - host passes xT/WT in bf16 (halves DMA, kills f32r staging copies)
  - per head-pair j: projection+normalization prep for pair j+1 overlaps
    scores+exp of pair j and PV of pair j-1, so the ACT exp stream
    (~29.5M exps/core, the hard floor) paces the kernel
  - k-side L2 norm folded into exp via per-partition scale AP:
    P = exp(s_raw * rinv_k) -- no k normalize multiply at all
  - q-side norm: gmat-reduce -> ACT sqrt -> reciprocal -> partition-
    broadcast DMA -> one in-place multiply
  - score matmuls for the head pair sit on disjoint 64-partition row
    groups -> concurrent PE execution
  - PV uses vaug (v + ones column) so the softmax denominator falls out
    of the PV matmul; eviction divide runs on the idle GPSIMD engine
    (normalize_recip) instead of DVE reciprocal+mult
"""

import os as _os
import sys

sys.path.insert(0, "/opt/trn_rl_repo")

from contextlib import ExitStack

import numpy as np
import ml_dtypes

import concourse.bass as bass
import concourse.tile as tile
from concourse import bacc, mybir
from concourse.bass_utils import run_bass_kernel_spmd

B, N, C = 8, 1568, 768
H, D = 12, 64
NC3 = 3 * C  # 2304
QK = 2 * C  # 1536
F32 = mybir.dt.float32
BF16 = mybir.dt.bfloat16
F8E3 = mybir.dt.float8e3

NT = 13  # token tiles of 128 (last is 32 wide)
KC = 6  # contraction chunks of 128 over C
QBLKS = [(0, 512), (512, 512), (1024, 512), (1536, 32)]
NHALF = 784

ADD = mybir.AluOpType.add
MULT = mybir.AluOpType.mult
DIV = mybir.AluOpType.divide
AF = mybir.ActivationFunctionType


def _knob(name, dflt):
    return _os.environ.get(name, dflt)


PT_DT = {"bf16": BF16, "f8e3": F8E3}[_knob("PT_DT", "f8e3")]
PVEVICT = _knob("PVEVICT", "gps")  # gps | dve
NORM = _knob("NORM", "lnexp")  # lnexp | newton


def _tw(t):
    return 32 if t == NT - 1 else 128


def _bcast_rows(src_row_ap, nrows, drop_first=True):
    """AP reading one partition row repeated nrows times (partition stride 0)."""
    ap = list(src_row_ap.ap)
    if drop_first:
        ap = ap[1:]
    return bass.AP(
        tensor=src_row_ap.tensor,
        offset=src_row_ap.offset,
        ap=[[0, nrows]] + ap,
    )


def _pin_act_table(arch):
    """Force Exp+Ln to resolve to the one table set containing both, so the
    kernel needs a single ACT_TABLE_LOAD instead of thrashing between the
    exp-only and ln-only sets (~1.5us per reload, mid-exp-stream)."""
    import concourse.hw_specs as hw_specs

    tabs = hw_specs.get_activation_tables(arch)  # cached by reference
    combined = "natural_log_exp_and_others"
    if combined not in tabs:
        return
    for name, fns in tabs.items():
        if name != combined:
            fns.discard(AF.Exp)
            fns.discard(AF.Ln)


def _build():
    nc = bacc.Bacc()
    _pin_act_table(nc.m.arch)
    xT = nc.dram_tensor("xT", [C, N], BF16, kind="ExternalInput")
    WT = nc.dram_tensor("WT", [C, NC3], BF16, kind="ExternalInput")
    bqkv = nc.dram_tensor("bqkv", [NC3], F32, kind="ExternalInput")
    out = nc.dram_tensor("out", [N, C], F32, kind="ExternalOutput")
    # per-query 1/|q| bounced through DRAM to broadcast across partitions
    scr = nc.dram_tensor("scr", [6, 2, N], F32, kind="Internal")

    with ExitStack() as ctx:
        tc = ctx.enter_context(tile.TileContext(nc))
        persist = ctx.enter_context(tc.tile_pool(name="persist", bufs=1))

        xb = persist.tile([128, KC, N], BF16)
        vaug = persist.tile([128, NT, H, D + 1], BF16)
        gmat = persist.tile([128, 2], BF16)  # per-head-half sumsq reducer
        bqk_t = persist.tile([128, 12], F32)  # q/k bias, per m-tile column
        bv_b = persist.tile([128, C], F32)  # v bias broadcast to all partitions

        for c in range(KC):
            nc.sync.dma_start(out=xb[:, c, :], in_=xT[c * 128 : (c + 1) * 128, :])
        nc.sync.dma_start(out=bqk_t, in_=bqkv[0:QK].rearrange("(t p) -> p t", p=128))
        bv_src = bqkv[QK:NC3]
        nc.vector.memset(gmat, 0.0)
        nc.vector.memset(gmat[0:64, 0:1], 1.0)
        nc.vector.memset(gmat[64:128, 1:2], 1.0)
        # ones in every head's extra column; v eviction overwrites the rest
        nc.vector.memset(vaug, 1.0)

        # ---------------- fused projection + attention ----------------
        with (
            tc.tile_pool(name="wv", bufs=1) as wvp,
            tc.tile_pool(name="wq", bufs=2) as wqp,
            tc.tile_pool(name="qk", bufs=2) as qkp,
            tc.tile_pool(name="sq", bufs=2) as sqp,
            tc.tile_pool(name="bc", bufs=1) as bcp,
            tc.tile_pool(name="rk", bufs=4) as rkp,
            tc.tile_pool(name="kss", bufs=2) as kssp,
            tc.tile_pool(name="pt", bufs=4 if PT_DT == F8E3 else 3) as ptp,
            tc.tile_pool(name="ot", bufs=2) as otp,
            tc.tile_pool(name="povs", bufs=3) as povsp,
            tc.tile_pool(name="sc", bufs=3, space="PSUM") as scp,
            tc.tile_pool(name="po", bufs=1, space="PSUM") as pop,
            tc.tile_pool(name="pj", bufs=1, space="PSUM") as pjp,
        ):
            wv = wvp.tile([128, KC, C], BF16)

            def emit_vproj(t):
                w = _tw(t)
                ps = scp.tile([128, NHALF], F32, tag="sc")
                for c in range(KC):
                    for (b0, bw) in ((0, 512), (512, 256)):
                        nc.tensor.matmul(
                            ps[0:w, b0 : b0 + bw],
                            xb[:, c, t * 128 : t * 128 + w],
                            wv[:, c, b0 : b0 + bw],
                            start=(c == 0),
                            stop=(c == KC - 1),
                        )
                nc.vector.tensor_add(
                    vaug[0:w, t, :, 0:D],
                    ps[0:w, 0:C].rearrange("p (h d) -> p h d", d=D),
                    bv_b[0:w, :].rearrange("p (h d) -> p h d", d=D),
                )
            state = {}

            def prep_steps(j):
                """Projection + normalization for head pair j; list of thunks."""
                st = {}
                steps = []

                def load_wq(m, key):
                    def f():
                        wq = wqp.tile([128, KC, 128], BF16, tag="wq", name=f"wq{m}")
                        nc.sync.dma_start(
                            out=wq,
                            in_=WT[:, m * 128 : (m + 1) * 128].rearrange(
                                "(c p) n -> p c n", p=128
                            ),
                        )
                        st[key] = wq

                    return f

                def proj_chunk(m, key, dstkey, q0, qw, blk, c):
                    # one matmul per step: prep trickles through the PE queue
                    # without ever blocking the score-psum refills ACT needs
                    def f():
                        if blk == 0 and c == 0:
                            st[dstkey] = qkp.tile(
                                [128, N], BF16, tag=dstkey[:2], name=f"{dstkey}{j}"
                            )
                        if c == 0:
                            st[dstkey + "ps"] = pjp.tile([128, 512], F32, tag="pj", name=f"{dstkey}ps{blk}")
                        nc.tensor.matmul(
                            st[dstkey + "ps"][:, 0:qw],
                            st[key][:, c, :],
                            xb[:, c, q0 : q0 + qw],
                            start=(c == 0),
                            stop=(c == KC - 1),
                        )

                    return f

                def proj_evict(m, dstkey, q0, qw):
                    def f():
                        # bias-add while evicting to bf16
                        nc.vector.tensor_scalar(
                            out=st[dstkey][:, q0 : q0 + qw],
                            in0=st[dstkey + "ps"][:, 0:qw],
                            scalar1=bqk_t[:, m : m + 1],
                            scalar2=None,
                            op0=ADD,
                        )

                    return f

                def proj_blk_steps(m, key, dstkey):
                    out = []
                    for blk, (q0, qw) in enumerate(QBLKS):
                        for c in range(KC):
                            out.append(proj_chunk(m, key, dstkey, q0, qw, blk, c))
                        out.append(proj_evict(m, dstkey, q0, qw))
                    return out

                def reduce_rinv(sqtile, dst):
                    # per-token per-head sumsq -> rinv = exp(-0.5*ln(ssq));
                    # Log+Exp share one ACT table set, so no table thrash
                    ps = pjp.tile([128, 512], F32, tag="pj")
                    for t in range(NT):
                        w = _tw(t)
                        nc.tensor.matmul(
                            ps[0:w, 2 * t : 2 * t + 2],
                            sqtile[:, t * 128 : t * 128 + w],
                            gmat,
                            start=True,
                            stop=True,
                        )
                    lg = kssp.tile([128, NT * 2], F32, tag="kss")
                    nc.scalar.activation(out=lg, in_=ps[:, 0 : 2 * NT], func=AF.Ln)
                    nc.scalar.activation(out=dst, in_=lg, func=AF.Exp, scale=-0.5)

                # ---- q side: m = j ----
                steps.append(load_wq(j, "wq_q"))
                steps.extend(proj_blk_steps(j, "wq_q", "qp"))

                def q_reduce_blk(q0, qw, blk):
                    # q-side rinv in [head-row, query] layout: ln on the sumsq
                    # psum chunk, one exp(-0.5*.) over the row at the end
                    def f():
                        if blk == 0:
                            st["sqq"] = sqp.tile(
                                [128, N], BF16, tag="sq", name=f"sqq{j}"
                            )
                            st["lgq"] = bcp.tile(
                                [2, N], F32, tag="lgq", name=f"lgq{j}"
                            )
                        nc.vector.tensor_mul(
                            st["sqq"][:, q0 : q0 + qw],
                            st["qp"][:, q0 : q0 + qw],
                            st["qp"][:, q0 : q0 + qw],
                        )
                        ps = pjp.tile([128, 512], F32, tag="pj")
                        nc.tensor.matmul(
                            ps[0:2, 0:qw],
                            gmat,
                            st["sqq"][:, q0 : q0 + qw],
                            start=True,
                            stop=True,
                        )
                        nc.scalar.activation(
                            out=st["lgq"][:, q0 : q0 + qw], in_=ps[0:2, 0:qw], func=AF.Ln
                        )
                        if blk == len(QBLKS) - 1:
                            rqT = bcp.tile([2, N], F32, tag="rqT", name=f"rqT{j}")
                            nc.scalar.activation(
                                out=rqT, in_=st["lgq"], func=AF.Exp, scale=-0.5
                            )
                            nc.sync.dma_start(out=scr[j], in_=rqT)

                    return f

                for blk, (q0, qw) in enumerate(QBLKS):
                    steps.append(q_reduce_blk(q0, qw, blk))

                def q_bcast():
                    bc = bcp.tile([128, N], F32, tag="bc", name=f"bc{j}")
                    for hb in range(2):
                        nc.sync.dma_start(
                            out=bc[64 * hb : 64 * hb + 64, :],
                            in_=_bcast_rows(scr[j, hb : hb + 1, :], 64),
                        )
                    st["bc"] = bc

                steps.append(q_bcast)

                def q_mul():
                    # in-place: qp becomes the normalized q
                    nc.vector.tensor_mul(st["qp"], st["qp"], st["bc"])

                steps.append(q_mul)

                # ---- k side: m = j + 6 ----
                steps.append(load_wq(j + 6, "wq_k"))
                steps.extend(proj_blk_steps(j + 6, "wq_k", "kp"))

                def k_sq_chunk(q0, qw, blk):
                    def f():
                        if blk == 0:
                            st["sqk"] = sqp.tile(
                                [128, N], BF16, tag="sq", name=f"sqk{j}"
                            )
                        nc.vector.tensor_mul(
                            st["sqk"][:, q0 : q0 + qw],
                            st["kp"][:, q0 : q0 + qw],
                            st["kp"][:, q0 : q0 + qw],
                        )

                    return f

                for blk, (q0, qw) in enumerate(QBLKS):
                    steps.append(k_sq_chunk(q0, qw, blk))

                def k_red_mm(t):
                    def f():
                        if t == 0:
                            st["kps"] = pjp.tile([128, 512], F32, tag="pj", name=f"kps{j}")
                        w = _tw(t)
                        nc.tensor.matmul(
                            st["kps"][0:w, 2 * t : 2 * t + 2],
                            st["sqk"][:, t * 128 : t * 128 + w],
                            gmat,
                            start=True,
                            stop=True,
                        )

                    return f

                for t in range(NT):
                    steps.append(k_red_mm(t))

                def k_rinv():
                    lg = kssp.tile([128, NT * 2], F32, tag="kss")
                    nc.scalar.activation(
                        out=lg, in_=st["kps"][:, 0 : 2 * NT], func=AF.Ln
                    )
                    rk = rkp.tile([128, NT, 2], F32, tag="rk", name=f"rk{j}")
                    nc.scalar.activation(
                        out=rk[:, :, :], in_=lg, func=AF.Exp, scale=-0.5
                    )
                    st["rk"] = rk

                steps.append(k_rinv)

                def finish():
                    state[j] = (st["qp"], st["kp"], st["rk"])

                steps.append(finish)
                return steps

            pts = {}

            def emit_scores(j, hb, t, half):
                # paired emission: the A-head (rows 0-63) and B-head (rows
                # 64-127) chunk matmuls are adjacent in the PE queue and run
                # concurrently on disjoint row groups
                qp, kp, rk = state[j][:3]
                w = _tw(t)
                base = half * NHALF
                scs = []
                for b in range(2):
                    sct = scp.tile(
                        [128, NHALF], F32, tag="sc", name=f"sc{j}_{t}_{half}_{b}"
                    )
                    scs.append(sct)
                for (b0, bw) in ((0, 512), (512, NHALF - 512)):
                    for b in range(2):
                        hp = 64 * b
                        nc.tensor.matmul(
                            scs[b][0:w, b0 : b0 + bw],
                            kp[hp : hp + 64, t * 128 : t * 128 + w],
                            qp[hp : hp + 64, base + b0 : base + b0 + bw],
                            start=True,
                            stop=True,
                        )
                # exp(s * rinv_k): cosine range needs no max subtraction
                for b in range(2):
                    nc.scalar.activation(
                        out=pts[2 * j + b][0:w, t, base : base + NHALF],
                        in_=scs[b][0:w, :],
                        func=AF.Exp,
                        scale=rk[0:w, t, b : b + 1],
                    )

            def emit_pv(j, hb, qt):
                qp, kp, rk = state[j][:3]
                q0 = qt * 128
                qw = _tw(qt)
                po = pop.tile([128, D + 1], F32, tag="po")
                for t in range(NT):
                    w = _tw(t)
                    nc.tensor.matmul(
                        po[0:qw, :],
                        pts[2 * j + hb][0:w, t, q0 : q0 + qw],
                        vaug[0:w, t, 2 * j + hb, :],
                        start=(t == 0),
                        stop=(t == NT - 1),
                    )
                if hb == 0 and qt == 0:
                    st_ot = otp.tile([128, NT, 2, D], F32, tag="ot", name=f"ot{j}")
                    state[j] = (qp, kp, rk, st_ot)
                ot = state[j][3]
                pv = povsp.tile([128, D + 1], F32, tag="povs")
                nc.vector.tensor_copy(pv[0:qw, :], po[0:qw, :])
                if PVEVICT == "gps":
                    nc.gpsimd.normalize_recip(
                        ot[0:qw, qt, hb, :],
                        pv[0:qw, 0:D],
                        pv[0:qw, D : D + 1],
                    )
                else:
                    linv = povsp.tile([128, 1], F32, tag="linv")
                    nc.vector.reciprocal(out=linv[0:qw, :], in_=pv[0:qw, D : D + 1])
                    nc.vector.tensor_scalar(
                        out=ot[0:qw, qt, hb, :],
                        in0=pv[0:qw, 0:D],
                        scalar1=linv[0:qw, :],
                        scalar2=None,
                        op0=MULT,
                    )
                if hb == 1:
                    nc.sync.dma_start(
                        out=out[q0 : q0 + qw, 2 * j * D : (2 * j + 2) * D],
                        in_=ot[0:qw, qt, :, :],
                    )

            # ---- schedule ----
            for s in prep_steps(0):
                s()
            # v weights/bias are not needed until iter-0's interleaved v-proj;
            # emitting their DMAs after prep(0)'s keeps ~5MB of transfers out
            # of the first exp's critical path
            nc.sync.dma_start(
                out=bv_b, in_=_bcast_rows(bv_src, 128, drop_first=False)
            )
            for c in range(KC):
                nc.sync.dma_start(
                    out=wv[:, c, :], in_=WT[c * 128 : (c + 1) * 128, QK:NC3]
                )
            for j in range(7):
                steps = prep_steps(j + 1) if j + 1 <= 5 else []
                per_t = (len(steps) + NT - 1) // NT if steps else 0
                si = 0
                if j < 6:
                    pts[2 * j] = ptp.tile([128, NT, N], PT_DT, tag="pt", name=f"pt{2*j}")
                    pts[2 * j + 1] = ptp.tile(
                        [128, NT, N], PT_DT, tag="pt", name=f"pt{2*j+1}"
                    )
                for t in range(NT):
                    # fine-grained interleave: PE revisits score matmuls every
                    # ~1us so the 2-buffer score psum pool keeps ACT streaming
                    # both score halves first: ACT gets a 4-exp run-ahead
                    # buffer that covers the PV/prep interlude on the PE FIFO
                    if j < 6:
                        emit_scores(j, 0, t, 0)
                        emit_scores(j, 0, t, 1)
                    if j == 0:
                        emit_vproj(t)
                    if j > 0:
                        emit_pv(j - 1, 0, t)
                        emit_pv(j - 1, 1, t)
                    for _ in range(per_t):
                        if si < len(steps):
                            steps[si]()
                            si += 1
                while si < len(steps):
                    steps[si]()
                    si += 1
                if j > 0:
                    del pts[2 * (j - 1)]
                    del pts[2 * (j - 1) + 1]
                    del state[j - 1]

    nc.compile()
    return nc


_PROGRAM = None


def _get_program():
    global _PROGRAM
    if _PROGRAM is None:
        _PROGRAM = _build()
    return _PROGRAM


_LAST_RESULTS = None


def kernel(x, W_qkv, b_qkv):
    global _LAST_RESULTS
    nc = _get_program()
    xT = np.ascontiguousarray(
        np.transpose(np.asarray(x, np.float32), (0, 2, 1))
    ).astype(ml_dtypes.bfloat16)
    WTh = np.ascontiguousarray(np.asarray(W_qkv, np.float32).T).astype(
        ml_dtypes.bfloat16
    )
    bh = np.ascontiguousarray(np.asarray(b_qkv, np.float32))
    in_maps = [{"xT": xT[b], "WT": WTh, "bqkv": bh} for b in range(B)]
    res = run_bass_kernel_spmd(nc, in_maps, core_ids=list(range(B)))
    _LAST_RESULTS = res
    o = np.stack([np.asarray(res.results[b]["out"]) for b in range(B)], axis=0)
    return np.ascontiguousarray(o.astype(np.float32))


if __name__ == "__main__":
    _build()
    print("build OK")
